# revision 14
# baseline (speedup 1.0000x reference)
"""Trainium2 Bass kernel for DiffeqSolver (fixed-grid RK4 over a tanh-MLP ODE).

reference:
  f(y) = tanh(y @ W1 + b1) @ W2 + b2        y: [B, D], W1: [D, H], W2: [H, D]
  63 RK4 steps over time_steps[64]; output pred_y [T=64, B=1024, D=512].

Strategy:
  - Data-parallel over batch: 8 cores x 128 rows each. No collectives.
  - All-feature-major on device: state y^T with D on partitions (4 chunks of
    128), batch (128) on the free dim. Both matmuls then use the weights as
    the stationary operand (lhsT) directly -- no activation transposes ever.
      h^T[m] = sum_c W1[c,m]^T @ u^T[c]     (32 matmuls, N=128)
      z^T[j] = sum_k W2[k,j]^T @ g^T[k]     (32 matmuls, N=128)
  - Matmul operands in fp16 (1 cycle/row on PE; fp32 would be 4). PSUM
    accumulation and the RK4 state/combines stay fp32. Empirically this
    yields ~1e-4 global relative error on this problem.
  - dt values and weight layouts are specialized on the host per call.
  - The RK4 combine uses an incremental p-chain (p_i = p_{i-1} + w_i dt/6 k_i)
    so the step boundary only waits on the last stage's z.
  - Output is DMA'd feature-major straight from the state tile (contiguous,
    no PE transposes); the host undoes the transpose when assembling pred_y.
"""

import os
import sys

import ml_dtypes
import numpy as np

if "/opt/trn_rl_repo" not in sys.path:
    sys.path.insert(0, "/opt/trn_rl_repo")

import concourse.bass as bass
import concourse.mybir as mybir
import concourse.tile as tile
from concourse import bacc
from concourse.bass_utils import run_bass_kernel_spmd

B, D, H, T = 1024, 512, 1024, 64
NCORES = 8
BP = B // NCORES          # 128 batch rows per core
DC = D // 128             # 4 D-chunks
HC = H // 128             # 8 H-chunks
NSTEP = T - 1

F32 = mybir.dt.float32
F16 = mybir.dt.float16
F8 = mybir.dt.float8e4   # TRN e4m3 (ml_dtypes.float8_e4m3, max normal 240)

# fp8 path: weights are scaled by WSCALE before e4m3 quantization (keeps all
# N(0,1/sqrt(K)) weights in the normal range); the 1/WSCALE compensation is
# folded into the tanh activation scale (MM1) and the RK4 combine scalars
# (MM2), so no extra ops are spent on it.
WSCALE = 32.0


def _build_program(dts, has_b1, has_b2, fp8=True, compile=True, reps=1, timing=False, ablate=(), fm_out=True):
    """Trace + compile the per-core SPMD program. dts: list of python floats.

    fp8=True: matmul operands in e4m3 with DoubleRow perf mode (two 128-row
    contraction chunks per PE instruction, 2x fp16 throughput). Weights are
    host-scaled by WSCALE; compensated via tanh scale and combine scalars.

    timing=True: outputs go to internal DRAM (not transferred) and the body
    repeats `reps` times in a HW loop -- for differential wall-clock timing."""
    nsteps = len(dts)
    mm_dtype = F8 if fp8 else F16
    DR = mybir.MatmulPerfMode.DoubleRow if fp8 else None
    nc = bacc.Bacc(
        "TRN2",
        target_bir_lowering=False,
        debug=False,
        enable_asserts=True,
        num_devices=NCORES,
    )

    if fp8:
        w1r = nc.dram_tensor("w1r", [128, DC, HC * 128], F8, kind="ExternalInput")
        w2r = nc.dram_tensor("w2r", [128, HC, DC * 128], F8, kind="ExternalInput")
        fp16d = nc.dram_tensor("fp16d", [128, DC, 128], F8, kind="ExternalInput")
    else:
        w1r = nc.dram_tensor("w1r", [128, DC * HC * 128], mm_dtype, kind="ExternalInput")
        w2r = nc.dram_tensor("w2r", [128, HC * DC * 128], mm_dtype, kind="ExternalInput")
        fp16d = nc.dram_tensor("fp16d", [128, D], mm_dtype, kind="ExternalInput")
    ident = nc.dram_tensor("ident", [128, 128], F32, kind="ExternalInput")
    fp32d = nc.dram_tensor("fp32d", [128, D], F32, kind="ExternalInput")
    if has_b1:
        b1d = nc.dram_tensor("b1c", [128, HC], F32, kind="ExternalInput")
    if has_b2:
        b2d = nc.dram_tensor("b2c", [128, DC], F32, kind="ExternalInput")
    if timing:
        tout_d = nc.dram_tensor("tout", [128, 4], F32, kind="ExternalOutput")
    else:
        out_d = nc.dram_tensor("yout", [nsteps, 128, D], F32, kind="ExternalOutput")

    AF = mybir.ActivationFunctionType
    OP = mybir.AluOpType

    with tile.TileContext(nc) as tc, tc.tile_pool(name="persist", bufs=1) as persist:
        # ---- persistent tiles -------------------------------------------
        if fp8:
            w1sb = persist.tile([128, DC, HC * 128], F8, tag="w1sb", name="w1sb")
            w2sb = persist.tile([128, HC, DC * 128], F8, tag="w2sb", name="w2sb")
            u0 = persist.tile([128, DC, 128], F8, tag="u0", name="u0")
        else:
            w1sb = persist.tile([128, DC * HC * 128], mm_dtype, tag="w1sb", name="w1sb")
            w2sb = persist.tile([128, HC * DC * 128], mm_dtype, tag="w2sb", name="w2sb")
            u0 = persist.tile([128, D], mm_dtype, tag="u0", name="u0")
        idsb = persist.tile([128, 128], F32, tag="idsb", name="idsb")
        yT = persist.tile([128, D], F32, tag="yT", name="yT")      # fp32 state
        if "tanh" in ablate:
            u0big = persist.tile(
                [128, HC, 128] if fp8 else [128, H], mm_dtype, tag="u0big", name="u0big"
            )
            touch = persist.tile([128, 64], F32, tag="touch", name="touch")
            nc.gpsimd.memset(u0big[:], 0.01)
        elif "dve" in ablate:
            touch = persist.tile([128, 64], F32, tag="touch", name="touch")
        if has_b1:
            b1sb = persist.tile([128, HC], F32, tag="b1sb", name="b1sb")
        if has_b2:
            b2sb = persist.tile([128, DC], F32, tag="b2sb", name="b2sb")

        nc.sync.dma_start(w1sb[:], w1r[:])
        nc.sync.dma_start(w2sb[:], w2r[:])
        nc.sync.dma_start(idsb[:], ident[:])
        nc.sync.dma_start(yT[:], fp32d[:])
        nc.sync.dma_start(u0[:], fp16d[:])
        if has_b1:
            nc.sync.dma_start(b1sb[:], b1d[:])
        if has_b2:
            nc.sync.dma_start(b2sb[:], b2d[:])

        with (
            tc.tile_pool(name="dram", bufs=1, space="DRAM") as dram_pool,
            tc.tile_pool(name="hps", bufs=2, space="PSUM") as hps_pool,
            tc.tile_pool(name="zps", bufs=4 if "zps4" in ablate else 3, space="PSUM") as zps_pool,
            tc.tile_pool(name="ybm", bufs=1, space="PSUM") as ybm_pool,
            tc.tile_pool(name="upool", bufs=3 if "sbuf3" in ablate else 2) as upool,
            tc.tile_pool(name="ppool", bufs=3 if "sbuf3" in ablate else 2) as ppool,
            tc.tile_pool(name="gpool", bufs=3 if "sbuf3" in ablate else 2) as gpool,
            tc.tile_pool(name="kts", bufs=2) as ktpool,
            tc.tile_pool(name="yo", bufs=3) as yopool,
        ):
            def w1chunk(c, m):
                s = (c * HC + m) * 128
                return w1sb[:, s : s + 128]

            def w2chunk(k, j):
                s = (k * DC + j) * 128
                return w2sb[:, s : s + 128]

            # fp8: tanh input is hps = WSCALE*h, compensate with scale=
            act_scale = (1.0 / WSCALE) if fp8 else 1.0

            def f_eval(u16):
                """u16: feature-major eval point ([128,DC,128] fp8 / [128,D] fp16).
                Returns zT psum tile [128, D] fp32 (= f(u) - b2, feature-major;
                scaled by WSCALE in the fp8 path)."""
                hps = hps_pool.tile([128, H], F32, tag="hps")
                for m in range(HC):
                    om = hps[:, m * 128 : (m + 1) * 128]
                    if fp8:
                        # DoubleRow: each instr contracts 2 D-chunks (256 rows)
                        for c in range(0, DC, 2):
                            nc.tensor.matmul(
                                om,
                                w1sb[:, c : c + 2, m * 128 : (m + 1) * 128],
                                u16[:, c : c + 2, :],
                                start=(c == 0),
                                stop=(c == DC - 2),
                                perf_mode=DR,
                            )
                    else:
                        for c in range(DC):
                            nc.tensor.matmul(
                                om,
                                w1chunk(c, m),
                                u16[:, c * 128 : (c + 1) * 128],
                                start=(c == 0),
                                stop=(c == DC - 1),
                            )
                if fp8:
                    gt = gpool.tile([128, HC, 128], F8, tag="gt")

                    def gsl(a, b):  # slice of gt covering h-chunks [a, b)
                        return gt[:, a:b, :]
                else:
                    gt = gpool.tile([128, H], mm_dtype, tag="gt")

                    def gsl(a, b):
                        return gt[:, a * 128 : b * 128]

                if "tanh" in ablate:
                    # timing-ablation: break the MM1->ACT->MM2 dependency; MM2
                    # streams from a static tile; touch hps so tiles release.
                    nc.vector.tensor_copy(touch[:, 0:8], hps[:, 0:1024:128])
                    gt = u0big
                elif has_b1:
                    for m in range(HC):
                        sl = slice(m * 128, (m + 1) * 128)
                        nc.scalar.activation(
                            gsl(m, m + 1), hps[:, sl], AF.Tanh,
                            bias=b1sb[:, m : m + 1], scale=act_scale,
                        )
                else:
                    # bank0 whole, bank1 split in two: MM2's last k-chunks
                    # wait on a 256-wide ACT op instead of 512 (A/B-measured
                    # win together with zps bufs=3)
                    nc.scalar.activation(gsl(0, 4), hps[:, :512], AF.Tanh, scale=act_scale)
                    nc.scalar.activation(gsl(4, 6), hps[:, 512:768], AF.Tanh, scale=act_scale)
                    nc.scalar.activation(gsl(6, 8), hps[:, 768:], AF.Tanh, scale=act_scale)
                if "dve" in ablate:
                    # timing-ablation: MM1 of every eval streams from u0
                    # (vector STT chain off the critical path entirely)
                    pass
                zps = zps_pool.tile([128, D], F32, tag="zps")
                for j in range(DC):
                    oj = zps[:, j * 128 : (j + 1) * 128]
                    if fp8:
                        for k in range(0, HC, 2):
                            nc.tensor.matmul(
                                oj,
                                w2sb[:, k : k + 2, j * 128 : (j + 1) * 128],
                                gt[:, k : k + 2, :],
                                start=(k == 0),
                                stop=(k == HC - 2),
                                perf_mode=DR,
                            )
                    else:
                        for k in range(HC):
                            nc.tensor.matmul(
                                oj,
                                w2chunk(k, j),
                                gt[:, k * 128 : (k + 1) * 128],
                                start=(k == 0),
                                stop=(k == HC - 1),
                            )
                return zps

            if timing:
                out_d = dram_pool.tile([nsteps, 128, D], F32, name="out_i")

            from contextlib import nullcontext

            def emit_output(t):
                if "output" in ablate:
                    return
                if fm_out:
                    # feature-major dump: contiguous DMA straight from the
                    # state tile; the host undoes the transpose. Saves the PE
                    # transposes + PSUM evacuation entirely.
                    nc.sync.dma_start(out_d[t], yT[:])
                    return
                # batch-major output for step t (reads yT as of end of step t):
                # 4 PE transposes -> PSUM, evacuate, DMA out. Emitted lazily
                # during step t+1 so it never stalls the PE at the boundary.
                ybm = ybm_pool.tile([128, D], F32, tag="ybm")
                for c in range(DC):
                    sl = slice(c * 128, (c + 1) * 128)
                    nc.tensor.transpose(ybm[:, sl], yT[:, sl], idsb[:])
                yo = yopool.tile([128, D], F32, tag="yo")
                nc.scalar.copy(yo[:], ybm[:])
                nc.sync.dma_start(out_d[t], yo[:])

            def new_u():
                if fp8:
                    return upool.tile([128, DC, 128], F8, tag="un", name="un")
                return upool.tile([128, D], mm_dtype, tag="un", name="un")

            def u_halves(un):
                # (first-half, second-half) views for the ustt2 split
                if fp8:
                    return (un[:, 0:2, :], un[:, 2:4, :])
                return (un[:, 0:256], un[:, 256:512])

            # fp8 (no b2): zps carries a WSCALE factor; fold 1/WSCALE into
            # the combine scalars. With b2 the kt pass descales instead.
            kscale = (1.0 / WSCALE) if (fp8 and not has_b2) else 1.0
            # GPSIMD/Pool cannot access PSUM on TRN2; every zps consumer must
            # be DVE (or ACT for scale-copy shapes).
            peng = nc.vector

            loop_ctx = tc.For_i(0, reps, 1) if reps > 1 else nullcontext()
            u_cur = u0
            with loop_ctx:
                for t in range(nsteps):
                    dt = dts[t]
                    # RK4: u_{i+1} = y + c_i k_i;  y' = y + dt/6 sum w_i k_i.
                    # Incremental p-chain: p_i = p_{i-1} + (w_i dt/6) k_i with
                    # p_0 = y, so the boundary only waits on the last z.
                    stage_c = [dt * 0.5 * kscale, dt * 0.5 * kscale, dt * kscale]
                    pw = [w * kscale for w in (dt / 6.0, dt / 3.0, dt / 3.0, dt / 6.0)]
                    p_prev = yT
                    for i in range(4):
                        zps = f_eval(u_cur)
                        if "dve" in ablate:
                            nc.vector.tensor_copy(touch[:, 8:12], zps[:, 0:512:128])
                            continue
                        if has_b2:
                            kt = ktpool.tile([128, D], F32, tag="kt")
                            for j in range(DC):
                                sl = slice(j * 128, (j + 1) * 128)
                                if fp8:
                                    nc.vector.tensor_scalar(
                                        kt[:, sl], zps[:, sl], 1.0 / WSCALE,
                                        b2sb[:, j : j + 1], OP.mult, OP.add,
                                    )
                                else:
                                    nc.vector.tensor_scalar_add(
                                        kt[:, sl], zps[:, sl], b2sb[:, j : j + 1]
                                    )
                            ksrc = kt
                        else:
                            ksrc = zps
                        if i < 3:
                            un = new_u()
                            if "ustt2" in ablate:
                                # first half reads z chunks 0-1 (ready at 50%
                                # of MM2, j-outer) -> runs under MM2's tail;
                                # only the 256-wide second half is exposed.
                                ua, ub = u_halves(un)
                                nc.vector.scalar_tensor_tensor(
                                    ua, ksrc[:, 0:256], stage_c[i], yT[:, 0:256], OP.mult, OP.add
                                )
                                nc.vector.scalar_tensor_tensor(
                                    ub, ksrc[:, 256:512], stage_c[i], yT[:, 256:512], OP.mult, OP.add
                                )
                            else:
                                nc.vector.scalar_tensor_tensor(
                                    un[:], ksrc[:], stage_c[i], yT[:], OP.mult, OP.add
                                )
                            u_cur = un
                            pn = ppool.tile([128, D], F32, tag="pn")
                            peng.scalar_tensor_tensor(
                                pn[:], ksrc[:], pw[i], p_prev[:], OP.mult, OP.add
                            )
                            p_prev = pn
                        else:
                            # y_{t+1} = p3 + (dt/6) k4: low-precision eval
                            # point for the next step (critical path) on DVE,
                            # fp32 state update on peng.
                            if t < nsteps - 1 or timing:
                                un = new_u()
                                if "ustt2" in ablate:
                                    ua, ub = u_halves(un)
                                    nc.vector.scalar_tensor_tensor(
                                        ua, ksrc[:, 0:256], pw[i], p_prev[:, 0:256], OP.mult, OP.add
                                    )
                                    nc.vector.scalar_tensor_tensor(
                                        ub, ksrc[:, 256:512], pw[i], p_prev[:, 256:512], OP.mult, OP.add
                                    )
                                else:
                                    nc.vector.scalar_tensor_tensor(
                                        un[:], ksrc[:], pw[i], p_prev[:], OP.mult, OP.add
                                    )
                                u_cur = un
                            peng.scalar_tensor_tensor(
                                yT[:], ksrc[:], pw[i], p_prev[:], OP.mult, OP.add
                            )
                        if i == 0 and t > 0:
                            # step t-1's output block, emitted mid-step so the
                            # PE transposes hide behind eval-1 matmuls (yT
                            # still holds y_t here; it's rewritten at i==3).
                            emit_output(t - 1)
                emit_output(nsteps - 1)

            if timing:
                dyo = yopool.tile([128, 4], F32, tag="dyo")
                nc.vector.tensor_copy(dyo[:], yT[:, 0:4])
                nc.sync.dma_start(tout_d[:], dyo[:])

    if compile:
        nc.compile()
    return nc


_cache = {}


def _make_in_maps(first_point, W1, b1, W2, b2, has_b1, has_b2, fp8=True):
    # host-side operand layouts
    if fp8:
        mmnp = ml_dtypes.float8_e4m3
        wmul = WSCALE
    else:
        mmnp = np.float16
        wmul = 1.0
    # W1 chunk (c,m) at free offset (c*HC+m)*128: w1r[p, (c*HC+m)*128+q] = W1[c*128+p, m*128+q]
    w1r = (
        np.ascontiguousarray(
            W1.reshape(DC, 128, HC, 128).transpose(1, 0, 2, 3).reshape(128, DC * HC * 128)
        )
        * wmul
    ).astype(mmnp)
    w2r = (
        np.ascontiguousarray(
            W2.reshape(HC, 128, DC, 128).transpose(1, 0, 2, 3).reshape(128, HC * DC * 128)
        )
        * wmul
    ).astype(mmnp)
    if fp8:
        w1r = w1r.reshape(128, DC, HC * 128)
        w2r = w2r.reshape(128, HC, DC * 128)
    ident = np.eye(128, dtype=np.float32)
    b1c = np.ascontiguousarray(b1.reshape(HC, 128).T).astype(np.float32)
    b2c = np.ascontiguousarray(b2.reshape(DC, 128).T).astype(np.float32)

    in_maps = []
    for i in range(NCORES):
        shard = first_point[i * BP : (i + 1) * BP]  # [128, 512]
        fpT = np.ascontiguousarray(
            shard.reshape(BP, DC, 128).transpose(2, 1, 0).reshape(128, D)
        )
        u0 = fpT.astype(mmnp)
        if fp8:
            u0 = u0.reshape(128, DC, 128)
        m = {
            "w1r": w1r,
            "w2r": w2r,
            "ident": ident,
            "fp32d": fpT.astype(np.float32),
            "fp16d": u0,
        }
        if has_b1:
            m["b1c"] = b1c
        if has_b2:
            m["b2c"] = b2c
        in_maps.append(m)
    return in_maps


def kernel(first_point, time_steps, W1, b1, W2, b2):
    first_point = np.asarray(first_point, dtype=np.float32)
    time_steps = np.asarray(time_steps, dtype=np.float32)
    W1 = np.asarray(W1, dtype=np.float32)
    b1 = np.asarray(b1, dtype=np.float32)
    W2 = np.asarray(W2, dtype=np.float32)
    b2 = np.asarray(b2, dtype=np.float32)

    dts = tuple(float(x) for x in (time_steps[1:] - time_steps[:-1]))
    has_b1 = bool(np.any(b1 != 0.0))
    has_b2 = bool(np.any(b2 != 0.0))
    # fp8/DoubleRow measured SLOWER end-to-end (1354us vs 817us): at free-dim
    # 128 DoubleRow serializes its 256-column LDWEIGHTS against the matmul
    # (it also disables FWL), so the PE win is only ~12%, while the shorter
    # PE stream exposes the ACT tanh chain (+1.0us/eval) and the DVE combine
    # tail (+1.5us/eval) that fp16's longer stream fully hides. The fp16
    # kernel runs at the PE port floor (~8192 col/eval, zero exposed
    # latency), so it stays the default.
    fp8 = bool(int(os.environ.get("KERNEL_FP8", "0")))

    key = (dts, has_b1, has_b2, fp8)
    if key not in _cache:
        _cache[key] = _build_program(list(dts), has_b1, has_b2, fp8=fp8)
    nc = _cache[key]

    in_maps = _make_in_maps(first_point, W1, b1, W2, b2, has_b1, has_b2, fp8=fp8)

    res = run_bass_kernel_spmd(
        nc,
        in_maps,
        core_ids=list(range(NCORES)),
        trace=bool(int(os.environ.get("KERNEL_TRACE", "0"))),
    )
    kernel._last_results = res

    out = np.empty((T, B, D), dtype=np.float32)
    out[0] = first_point
    for i in range(NCORES):
        dump = res.results[i]["yout"]  # [nsteps, 128(p), D] feature-major
        ns = dump.shape[0]
        # dump[t, p, c*128+b] = y[b, c*128+p]  ->  [t, b, c*128+p]
        out[1:, i * BP : (i + 1) * BP, :] = (
            dump.reshape(ns, 128, DC, 128).transpose(0, 3, 2, 1).reshape(ns, BP, D)
        )
    return out

